# revision 4
# baseline (speedup 1.0000x reference)
"""Trainium2 Bass kernel for Grover2UnimolEmbedding (gnn_message_passing).

Strategy (data-parallel over molecules, 16 molecules per core x 8 cores):
  - Host "inspector": reads the tiny int index tensors (a_scope/b_scope/b2a/
    b2revb), gathers per-molecule padded atom/bond feature slabs, transposes
    them (so the device needs no PE transposes), and builds one-hot coordinate
    tables with last-write-wins dedupe matching jax-CPU scatter semantics.
  - Device per molecule:
      atoms:  [128,512] = xaT.T @ W_atom  (+ valid-masked bias) as fp32r
              matmuls (full-rate fp32 on trn2), streamed to atoms_emb.
      bonds:  [256,16] bond-head embeddings likewise.
      apairs: sparse scatter expressed as one-hot matmuls: for each group of
              4 heads, PSUM[i, h*128+j] = onehot_i.T @ (onehot_j * val_h),
              initialized with the -inf padding-column mask via a k=1 matmul.
              Exact because each output cell gets at most one contribution.
"""

import numpy as np

B = 128          # molecules
NH = 16          # heads
NA = 128         # padded atoms per molecule
NBM = 256        # padded bonds per molecule
D = 512          # model dim (2*256 concat)
NCORES = 8
MPC = B // NCORES  # molecules per core
NEG_INF = np.float32(-np.inf)

_PROG = None  # cached compiled program


# --------------------------------------------------------------------------
# Device program
# --------------------------------------------------------------------------
def _build_program():
    global _PROG
    if _PROG is not None:
        return _PROG

    import concourse.bass as bass
    import concourse.mybir as mybir
    from concourse import bacc
    from concourse import tile

    dt = mybir.dt
    f32 = dt.float32
    f32r = dt.float32r
    EQ = mybir.AluOpType.is_equal

    nc = bacc.Bacc(
        "TRN2", target_bir_lowering=False, debug=False, num_devices=NCORES
    )

    # ---- DRAM I/O (per-core shapes) ----
    xaT_d = nc.dram_tensor("xaT", [D, MPC * NA], f32r, kind="ExternalInput").ap()
    xbT_d = nc.dram_tensor("xbT", [D, MPC * NBM], f32r, kind="ExternalInput").ap()
    wa_d = nc.dram_tensor("wa", [D, D], f32r, kind="ExternalInput").ap()
    wb_d = nc.dram_tensor("wb", [D, NH], f32r, kind="ExternalInput").ap()
    ba_d = nc.dram_tensor("ba", [1, D], f32r, kind="ExternalInput").ap()
    bb_d = nc.dram_tensor("bb", [1, NH], f32r, kind="ExternalInput").ap()
    ik_d = nc.dram_tensor("ik", [128, MPC * 2], f32, kind="ExternalInput").ap()
    jk_d = nc.dram_tensor("jk", [128, MPC * 2], f32, kind="ExternalInput").ap()
    mask_d = nc.dram_tensor("mask4", [1, MPC * 512], f32r, kind="ExternalInput").ap()
    valid_d = nc.dram_tensor("valid", [1, MPC * NA], f32r, kind="ExternalInput").ap()
    iota_d = nc.dram_tensor("iota", [128, 128], f32, kind="ExternalInput").ap()
    ones_d = nc.dram_tensor("ones", [1, 128], f32r, kind="ExternalInput").ap()

    ae_d = nc.dram_tensor("ae", [NA, MPC, D], f32, kind="ExternalOutput").ap()
    apo_d = nc.dram_tensor(
        "apo", [MPC, NH, NA, NA], f32, kind="ExternalOutput"
    ).ap()

    NG = 4             # molecule groups
    GM = MPC // NG     # molecules per group

    with tile.TileContext(nc) as tc:
        with (
            tc.tile_pool(name="const", bufs=1) as cpool,
            tc.tile_pool(name="xa", bufs=NG) as xapool,
            tc.tile_pool(name="xb", bufs=NG) as xbpool,
            tc.tile_pool(name="work", bufs=3) as wp,
            tc.tile_pool(name="outb", bufs=4) as op,
            tc.tile_pool(name="psA", bufs=2, space="PSUM") as psA,
            tc.tile_pool(name="psB", bufs=2, space="PSUM") as psB,
            tc.tile_pool(name="psS", bufs=3, space="PSUM") as psS,
        ):
            # ---- constants ----
            wa_sb = cpool.tile([128, 4 * D], f32r)       # [k_part, (kt, col)]
            for kt in range(4):
                nc.sync.dma_start(
                    wa_sb[:, kt * D : (kt + 1) * D],
                    wa_d[kt * 128 : (kt + 1) * 128, :],
                )
            wb_sb = cpool.tile([128, 4 * NH], f32r)
            for kt in range(4):
                nc.sync.dma_start(
                    wb_sb[:, kt * NH : (kt + 1) * NH],
                    wb_d[kt * 128 : (kt + 1) * 128, :],
                )
            ba_sb = cpool.tile([1, D], f32r)
            nc.sync.dma_start(ba_sb[:], ba_d[:])
            bb_sb = cpool.tile([1, NH], f32r)
            nc.sync.dma_start(bb_sb[:], bb_d[:])
            ik_sb = cpool.tile([128, MPC * 2], f32)
            nc.sync.dma_start(ik_sb[:], ik_d[:])
            jk_sb = cpool.tile([128, MPC * 2], f32)
            nc.sync.dma_start(jk_sb[:], jk_d[:])
            mask_sb = cpool.tile([1, MPC * 512], f32r)
            nc.sync.dma_start(mask_sb[:], mask_d[:])
            valid_sb = cpool.tile([1, MPC * NA], f32r)
            nc.sync.dma_start(valid_sb[:], valid_d[:])
            iota_sb = cpool.tile([128, 128], f32)
            nc.sync.dma_start(iota_sb[:], iota_d[:])
            ones_sb = cpool.tile([1, 128], f32r)
            nc.sync.dma_start(ones_sb[:], ones_d[:])

            xa_tiles = {}
            xb_tiles = {}
            for g in range(NG):
                # [k_part, (kt, mol, col)] slabs for this molecule group
                xa_g = xapool.tile([128, 4 * GM * NA], f32r, tag="xa")
                for kt in range(4):
                    nc.sync.dma_start(
                        xa_g[:, kt * GM * NA : (kt + 1) * GM * NA],
                        xaT_d[kt * 128 : (kt + 1) * 128,
                              g * GM * NA : (g + 1) * GM * NA],
                    )
                xb_g = xbpool.tile([128, 4 * GM * NBM], f32r, tag="xb")
                for kt in range(4):
                    nc.sync.dma_start(
                        xb_g[:, kt * GM * NBM : (kt + 1) * GM * NBM],
                        xbT_d[kt * 128 : (kt + 1) * 128,
                              g * GM * NBM : (g + 1) * GM * NBM],
                    )
                xa_tiles[g] = xa_g
                xb_tiles[g] = xb_g

            scale_rr = 0  # round-robin over engines for the scaled copies

            for m in range(MPC):
                g, lm = divmod(m, GM)
                xa_g = xa_tiles[g]
                xb_g = xb_tiles[g]

                # ---- bond-head embeddings: [256 bonds, 16 heads] ----
                psb = psB.tile([128, 2 * NH], f32)
                for rt in range(2):
                    out_ap = psb[:, rt * NH : (rt + 1) * NH]
                    nc.tensor.matmul(
                        out_ap,
                        ones_sb[:],
                        bb_sb[:],
                        start=True, stop=False,
                    )
                    for kt in range(4):
                        lhs = xb_g[:, kt * GM * NBM + lm * NBM + rt * 128 :
                                   kt * GM * NBM + lm * NBM + rt * 128 + 128]
                        nc.tensor.matmul(
                            out_ap,
                            lhs,
                            wb_sb[:, kt * NH : (kt + 1) * NH],
                            start=False, stop=(kt == 3),
                        )
                bv = wp.tile([128, 2 * NH], f32, tag="bv")
                nc.vector.tensor_copy(bv[:], psb[:])

                # ---- one-hot matrices for this molecule ----
                oj = wp.tile([128, 256], f32, tag="oj")
                oi = wp.tile([128, 256], f32r, tag="oi")
                for rt in range(2):
                    nc.vector.tensor_scalar(
                        oj[:, rt * 128 : (rt + 1) * 128], iota_sb[:],
                        jk_sb[:, m * 2 + rt : m * 2 + rt + 1], None, op0=EQ,
                    )
                    nc.gpsimd.tensor_scalar(
                        oi[:, rt * 128 : (rt + 1) * 128], iota_sb[:],
                        ik_sb[:, m * 2 + rt : m * 2 + rt + 1], None, op0=EQ,
                    )

                # ---- atom embeddings: [128 pos, 512] ----
                psa = psA.tile([128, D], f32)
                nc.tensor.matmul(
                    psa[:],
                    valid_sb[0:1, m * NA : (m + 1) * NA],
                    ba_sb[:],
                    start=True, stop=False,
                )
                for kt in range(4):
                    lhs = xa_g[:, kt * GM * NA + lm * NA :
                               kt * GM * NA + lm * NA + 128]
                    nc.tensor.matmul(
                        psa[:],
                        lhs,
                        wa_sb[:, kt * D : (kt + 1) * D],
                        start=False, stop=(kt == 3),
                    )
                ae_sb = op.tile([128, D], f32, tag="ae")
                nc.scalar.copy(ae_sb[:], psa[:])
                nc.sync.dma_start(ae_d[:, m, :], ae_sb[:])

                # ---- apairs scatter, 4 heads per pass ----
                for hg in range(4):
                    rhs4 = wp.tile([128, 2 * 512], f32r, tag="rhs4")
                    for rt in range(2):
                        for hl in range(4):
                            h = hg * 4 + hl
                            dst = rhs4[:, rt * 512 + hl * 128 :
                                       rt * 512 + hl * 128 + 128]
                            src = oj[:, rt * 128 : (rt + 1) * 128]
                            val = bv[:, rt * NH + h : rt * NH + h + 1]
                            e = scale_rr % 3
                            scale_rr += 1
                            if e == 0:
                                nc.vector.tensor_scalar_mul(dst, src, val)
                            elif e == 1:
                                nc.gpsimd.tensor_scalar_mul(dst, src, val)
                            else:
                                nc.scalar.mul(dst, src, val)
                    pss = psS.tile([128, 512], f32)
                    nc.tensor.matmul(
                        pss[:],
                        ones_sb[:],
                        mask_sb[0:1, m * 512 : (m + 1) * 512],
                        start=True, stop=False,
                    )
                    for rt in range(2):
                        nc.tensor.matmul(
                            pss[:],
                            oi[:, rt * 128 : (rt + 1) * 128],
                            rhs4[:, rt * 512 : (rt + 1) * 512],
                            start=False, stop=(rt == 1),
                        )
                    osb = op.tile([128, 512], f32, tag="osb")
                    if hg % 2 == 0:
                        nc.vector.tensor_copy(osb[:], pss[:])
                    else:
                        nc.scalar.copy(osb[:], pss[:])
                    nc.sync.dma_start(
                        apo_d[m, hg * 4 : (hg + 1) * 4, :, :].rearrange(
                            "h i j -> i h j"
                        ),
                        osb[:].rearrange("p (h j) -> p h j", h=4),
                    )

    nc.compile()
    _PROG = nc
    return nc


# --------------------------------------------------------------------------
# Host inspector: index preprocessing + per-core input maps
# --------------------------------------------------------------------------
def _to_f32r(a):
    """Round fp32 to the FP32R grid: 11-bit mantissa, low 12 bits zero (RNE)."""
    a = np.ascontiguousarray(a, np.float32)
    u = a.view(np.uint32)
    special = (u & np.uint32(0x7F800000)) == np.uint32(0x7F800000)  # inf/nan
    r = (u + np.uint32(0x7FF) + ((u >> np.uint32(12)) & np.uint32(1))) & np.uint32(0xFFFFF000)
    out = np.where(special, u, r).view(np.float32)
    return np.ascontiguousarray(out)


def _prepare_inputs(f_atoms, f_bonds, f_atoms_out, f_bonds_out, b2a, b2revb,
                    a_scope, b_scope, W_atom, b_atom, W_bond, b_bond):
    f_atoms = np.ascontiguousarray(f_atoms, np.float32)
    f_bonds = np.ascontiguousarray(f_bonds, np.float32)
    f_atoms_out = np.ascontiguousarray(f_atoms_out, np.float32)
    f_bonds_out = np.ascontiguousarray(f_bonds_out, np.float32)
    b2a = np.asarray(b2a).astype(np.int64)
    b2revb = np.asarray(b2revb).astype(np.int64)
    a_scope = np.asarray(a_scope).astype(np.int64)
    b_scope = np.asarray(b_scope).astype(np.int64)

    starts_a, lens = a_scope[:, 0], a_scope[:, 1]
    starts_b, nbs = b_scope[:, 0], b_scope[:, 1]
    Nb = f_bonds.shape[0]

    pos = np.arange(NA)
    valid = pos[None, :] < lens[:, None]                      # [B, NA]
    gidx = np.where(valid, starts_a[:, None] + pos[None, :], 0)
    xa_full = np.concatenate([f_atoms, f_atoms_out], axis=1)  # [Na, 512]
    xa_pad = xa_full[gidx] * valid[..., None]                 # [B, NA, 512]

    posb = np.arange(NBM)
    validb = posb[None, :] < nbs[:, None]
    gidxb = np.where(validb, starts_b[:, None] + posb[None, :], 0)
    xb_full = np.concatenate([f_bonds, f_bonds_out], axis=1)
    xb_pad = xb_full[gidxb] * validb[..., None]               # [B, NBM, 512]

    # scatter coordinates, matching reference's flat[idx] = bonds (last wins)
    ks = np.arange(Nb)
    mol = np.searchsorted(starts_b, ks, side="right") - 1
    molw = np.maximum(mol, 0)
    i_co = b2a[b2revb]
    j_co = b2a
    local = ks - starts_b[molw]
    # padding bond 0 scatters into molecule 0; give it a free pad slot
    slot0 = int(nbs[0])
    local[0] = slot0
    xb_pad[0, slot0] = xb_full[0]
    # last-write-wins dedupe over (mol, i, j)
    flat_key = (molw * NA + i_co) * NA + j_co
    order = np.arange(Nb)
    # np.unique keeps first occurrence -> reverse so last occurrence wins
    _, first_idx = np.unique(flat_key[::-1], return_index=True)
    keep = np.zeros(Nb, bool)
    keep[Nb - 1 - first_idx] = True

    ik_tab = np.full((B, NBM), -1.0, np.float32)
    jk_tab = np.full((B, NBM), -1.0, np.float32)
    ik_tab[molw[keep], local[keep]] = i_co[keep]
    jk_tab[molw[keep], local[keep]] = j_co[keep]

    maskrow = np.where(valid, np.float32(0.0), NEG_INF).astype(np.float32)
    mask4 = np.tile(maskrow, (1, 4))                          # [B, 512]
    valid_f = valid.astype(np.float32)
    iota_t = np.tile(np.arange(128, dtype=np.float32), (128, 1))
    iota_t = np.ascontiguousarray(iota_t)

    wa = _to_f32r(W_atom)
    wb = _to_f32r(W_bond)
    ba = _to_f32r(np.asarray(b_atom, np.float32).reshape(1, D))
    bb = _to_f32r(np.asarray(b_bond, np.float32).reshape(1, NH))

    in_maps = []
    for c in range(NCORES):
        sl = slice(c * MPC, (c + 1) * MPC)
        xaT = _to_f32r(xa_pad[sl].reshape(MPC * NA, D).T)     # [512, 2048]
        xbT = _to_f32r(xb_pad[sl].reshape(MPC * NBM, D).T)    # [512, 4096]
        ik_t = np.ascontiguousarray(
            ik_tab[sl].reshape(MPC, 2, 128).transpose(2, 0, 1).reshape(128, MPC * 2))
        jk_t = np.ascontiguousarray(
            jk_tab[sl].reshape(MPC, 2, 128).transpose(2, 0, 1).reshape(128, MPC * 2))
        in_maps.append({
            "xaT": xaT, "xbT": xbT, "wa": wa, "wb": wb, "ba": ba, "bb": bb,
            "ik": ik_t, "jk": jk_t,
            "mask4": _to_f32r(mask4[sl].reshape(1, MPC * 512)),
            "valid": _to_f32r(valid_f[sl].reshape(1, MPC * NA)),
            "iota": iota_t,
            "ones": np.ones((1, 128), np.float32),
        })
    return in_maps, valid


# --------------------------------------------------------------------------
# Public entry point
# --------------------------------------------------------------------------
def run(inputs: dict, trace: bool = False):
    from concourse import bass_utils

    kw = {k: v for k, v in inputs.items() if k != "n_atom"}
    in_maps, valid = _prepare_inputs(**kw)
    nc = _build_program()
    res = bass_utils.run_bass_kernel_spmd(
        nc, in_maps, core_ids=list(range(NCORES)), trace=trace
    )
    outs = res.results
    atoms_emb = np.concatenate([outs[c]["ae"] for c in range(NCORES)], axis=1)
    apairs = np.concatenate([outs[c]["apo"] for c in range(NCORES)], axis=0)
    padding_mask = ~valid
    return (atoms_emb, apairs, padding_mask), res


def kernel(**inputs):
    (atoms_emb, apairs, padding_mask), _ = run(inputs, trace=False)
    return atoms_emb, apairs, padding_mask
